# revision 1
# baseline (speedup 1.0000x reference)
"""Trainium2 Bass kernel for AnchorGNNPocket (GNN message passing).

Data-parallel over batch B=8: one complex per NeuronCore (forward only, no
collectives). Each core runs the full 4-layer GCL stack on its sample: dense
256x256 edge MLP, attention gating, masked scatter-sum, node MLP, output head.

Host precomputes input-derived constants only (embedding h0, pairwise d2,
adjacency logit mask, weight packs); all h-dependent compute runs on device.

Layout/scheduling (arrived at via neuron-profile iteration, 4.36ms -> 1.21ms):
- Edge tensors live as [H=128 partitions, j free]; edge rows processed in
  PAIRS ([128, 512] tiles = exactly one PSUM bank), emitted stage-by-stage
  over groups of 3 pairs so every engine sees batches of independent work.
- All edge matmuls are bf16 (fp32 matmul is 2-pass LOW_HIGH on trn2). The
  wc (x) d2[i,:] term keeps fp32-class accuracy via a K=3 bf16 hi/lo split:
  wc_hi@d2_hi + wc_hi@d2_lo + wc_lo@d2_hi.
- d2/madj rows are packed on partitions {0,32,64} (+1,+2 for the hi/lo split)
  because matmul APs must start at base partition 0/32/64 with lhsT and rhs
  bases equal.
- The per-row ha bias rides the DVE scalar_tensor_tensor that also adds hb
  (per-partition scalar operand), so the following relu is pair-level; the
  relu alternates between ScalarE and VectorE to balance engine load.
- Wat is replicated into all 128 stationary columns, so the attention matmul
  broadcasts att to every PSUM partition for free; adjacency+mask enter as a
  -1e9 logit row (K=1 accumulate; sigmoid of it vanishes). Gating+aggregation
  fuse into one DVE scalar_tensor_tensor per row: (m * 1/NORM) * sigmoid,
  accumulated into agg[:, i]. (Accumulation groups with >= 2 K=1 matmuls
  crash HW; one K=128 + one K=1 per group is the proven-safe shape.)

Per edge-row i the math is
  pre      = wc (x) d2[i,:] + hb + ha[:,i]
  m        = relu(We2^T relu(pre) + be2)
  att      = Wat^T m + madj[i,:] + bat
  agg[:,i] = sum_j (m / NORM) * sigmoid(att)
  (then per layer: h += relu([h, agg] @ Wn1 + bn1) @ Wn2 + bn2)
"""

import os
import sys

import numpy as np

if not any(os.path.isdir(os.path.join(p, "concourse")) for p in sys.path if p):
    sys.path.insert(0, "/opt/trn_rl_repo")

# ---- problem constants (hardcoded per contest rules) ----
B, NS, NP = 8, 32, 224
N = NS + NP                      # 256 nodes
LIG_NF, POK_NF, JNF, HID, OUT_NF, NLAYERS = 10, 25, 32, 128, 128, 4
CUT2 = 4.5 ** 2
NORM = 100.0

_F32 = np.float32
NPAIR = N // 2                   # 128 row-pairs
_NB = (NPAIR + 2) // 3           # pair slots per base partition (43)


def _np_silu(x):
    return x / (1.0 + np.exp(-x))


def _host_prep(inputs):
    """Host-side preprocessing: embedding h0, pairwise d2, adjacency logits."""
    x = np.concatenate([inputs["mol_x"], inputs["pocket_x"]], axis=1).astype(_F32)
    mask = np.concatenate([inputs["node_mask"], inputs["pocket_mask"]], axis=1).astype(
        _F32
    )
    hm = _np_silu(inputs["mol_h"].astype(_F32) @ inputs["W_mol"] + inputs["b_mol"])
    hp = _np_silu(
        inputs["pocket_h"].astype(_F32) @ inputs["W_pok"] + inputs["b_pok"]
    )
    h0 = (
        np.concatenate([hm, hp], axis=1) @ inputs["W_emb"] + inputs["b_emb"]
    ).astype(_F32)  # [B, N, H]

    diff = x[:, :, None, :] - x[:, None, :, :]
    d2 = np.sum(diff * diff, axis=-1, dtype=_F32)  # [B, N, N]
    idx = np.arange(N)
    lig_pair = (idx[:, None] < NS) & (idx[None, :] < NS)
    adj = np.where(lig_pair, 1.0, (d2 <= CUT2).astype(_F32))
    adj = adj * mask[:, :, None] * mask[:, None, :]
    madj = np.where(adj > 0, 0.0, -1.0e9).astype(_F32)
    return h0, d2, madj, mask


def _pack_pairs3(mat, dtype):
    """[256, 256] -> [65, 43*512]: row pair (2t, 2t+1) at partition 32*(t%3),
    cols (t//3)*512. Matmul rhs APs must start at partition 0/32/64."""
    out = np.zeros((65, _NB * 2 * N), dtype=dtype)
    for t in range(NPAIR):
        c = (t // 3) * 2 * N
        out[32 * (t % 3), c : c + N] = mat[2 * t]
        out[32 * (t % 3), c + N : c + 2 * N] = mat[2 * t + 1]
    return out


def _pack_d2_split(d2):
    """bf16 hi/lo split of d2 rows: partition b holds d2_hi, b+1 d2_lo,
    b+2 d2_hi again (pairs with lhsT rows [wc_hi, wc_hi, wc_lo])."""
    import ml_dtypes

    bf = ml_dtypes.bfloat16
    out = np.zeros((67, _NB * 2 * N), dtype=bf)
    for t in range(NPAIR):
        b, c = 32 * (t % 3), (t // 3) * 2 * N
        row = np.concatenate([d2[2 * t], d2[2 * t + 1]]).astype(_F32)
        hi = row.astype(bf)
        lo = (row - hi.astype(_F32)).astype(bf)
        out[b, c : c + 2 * N] = hi
        out[b + 1, c : c + 2 * N] = lo
        out[b + 2, c : c + 2 * N] = hi
    return out


def _pack_wc_split(wcr):
    """[1, L*H] f32 -> [67, L*H] bf16 with rows [wc_hi, wc_hi, wc_lo] at each
    base partition in {0, 32, 64}."""
    import ml_dtypes

    bf = ml_dtypes.bfloat16
    hi = wcr.astype(bf)
    lo = (wcr.astype(_F32) - hi.astype(_F32)).astype(bf)
    out = np.zeros((67, wcr.shape[-1]), dtype=bf)
    for b in (0, 32, 64):
        out[b] = hi
        out[b + 1] = hi
        out[b + 2] = lo
    return out


def _rep3(row):
    out = np.zeros((65, row.shape[-1]), dtype=row.dtype)
    out[0] = out[32] = out[64] = row
    return out


# weight-pack column offsets (per layer stride)
_PL = 6 * HID + 8  # wa, wb, We2, Wn1a, Wn1b, Wn2 (128 each) + small cols
_W_COLS = NLAYERS * _PL + HID + 8  # + W_out + wlin/bout/blin
_PLB = 2 * HID  # bf16 pack per layer: We2 | WatFull (Wat in all 128 cols)
_WB_COLS = NLAYERS * _PLB


def _pack_weights(inputs):
    import ml_dtypes

    wp = np.zeros((HID, _W_COLS), dtype=_F32)
    wcr = np.zeros((1, NLAYERS * HID), dtype=_F32)
    wpb = np.zeros((HID, _WB_COLS), dtype=ml_dtypes.bfloat16)
    We1 = inputs["We1"].astype(_F32)
    for l in range(NLAYERS):
        o = l * _PL
        wp[:, o : o + HID] = We1[l, :HID, :]              # wa
        wp[:, o + HID : o + 2 * HID] = We1[l, HID : 2 * HID, :]  # wb
        wp[:, o + 2 * HID : o + 3 * HID] = inputs["We2"][l]
        wp[:, o + 3 * HID : o + 4 * HID] = inputs["Wn1"][l][:HID, :]
        wp[:, o + 4 * HID : o + 5 * HID] = inputs["Wn1"][l][HID:, :]
        wp[:, o + 5 * HID : o + 6 * HID] = inputs["Wn2"][l]
        c = o + 6 * HID
        wp[:, c + 1] = inputs["be1"][l]
        wp[:, c + 2] = inputs["be2"][l]
        wp[:, c + 3] = inputs["bn1"][l]
        wp[:, c + 4] = inputs["bn2"][l]
        wp[:, c + 5] = inputs["bat"][l][0]                # bat replicated
        wcr[0, l * HID : (l + 1) * HID] = We1[l, 2 * HID, :]
        ob = l * _PLB
        wpb[:, ob : ob + HID] = inputs["We2"][l]
        wpb[:, ob + HID : ob + 2 * HID] = np.repeat(
            inputs["Wat"][l].astype(_F32), HID, axis=1
        )
    o = NLAYERS * _PL
    wp[:, o : o + HID] = inputs["W_out"].astype(_F32)
    wp[:, o + HID] = inputs["W_lin"][:, 0]
    wp[:, o + HID + 1] = inputs["b_out"]
    wp[0, o + HID + 2] = inputs["b_lin"][0]
    return wp, wcr, wpb


def _build(nc, tile_mod, bass_mod, n_layers, n_i):
    """Trace the per-core kernel into nc (a Bacc)."""
    mybir = __import__("concourse.mybir", fromlist=["mybir"])
    dt = mybir.dt.float32
    bf = mybir.dt.bfloat16
    AF = mybir.ActivationFunctionType
    ALU = mybir.AluOpType
    N2 = 2 * N

    assert n_i % 2 == 0
    npair = n_i // 2

    hT_d = nc.dram_tensor("hT0", [HID, N], dt, kind="ExternalInput")
    d2_d = nc.dram_tensor("d2p", [67, _NB * N2], bf, kind="ExternalInput")
    ma_d = nc.dram_tensor("adjp", [65, _NB * N2], bf, kind="ExternalInput")
    wp_d = nc.dram_tensor("wpack", [HID, _W_COLS], dt, kind="ExternalInput")
    wc_d = nc.dram_tensor("wcrows", [67, NLAYERS * HID], bf, kind="ExternalInput")
    wb_d = nc.dram_tensor("wpackb", [HID, _WB_COLS], bf, kind="ExternalInput")
    out_d = nc.dram_tensor("out", [1, NS], dt, kind="ExternalOutput")

    with tile_mod.TileContext(nc) as tc:
        with (
            tc.tile_pool(name="const", bufs=1) as cpool,
            tc.tile_pool(name="layer", bufs=2) as lpool,
            tc.tile_pool(name="work", bufs=4) as wpool,
            tc.tile_pool(name="psA", bufs=3, space="PSUM") as psA,
            tc.tile_pool(name="psB", bufs=3, space="PSUM") as psB,
            tc.tile_pool(name="psD", bufs=2, space="PSUM") as psD,
        ):
            # ---- load constants ----
            hT = cpool.tile([HID, N], dt, tag="hT0")
            d2p = cpool.tile([67, _NB * N2], bf, tag="d2p")
            adjp = cpool.tile([65, _NB * N2], bf, tag="adjp")
            wp = cpool.tile([HID, _W_COLS], dt, tag="wpack")
            wcr = cpool.tile([67, NLAYERS * HID], bf, tag="wcrows")
            wpb = cpool.tile([HID, _WB_COLS], bf, tag="wpackb")
            nc.sync.dma_start(hT[:], hT_d.ap())
            nc.sync.dma_start(d2p[:], d2_d.ap())
            nc.sync.dma_start(adjp[:], ma_d.ap())
            nc.sync.dma_start(wp[:], wp_d.ap())
            nc.sync.dma_start(wcr[:], wc_d.ap())
            nc.sync.dma_start(wpb[:], wb_d.ap())
            ones1_65 = cpool.tile([65, HID], bf, tag="ones1")
            nc.vector.memset(ones1_65[:], 1.0)

            hT_cur = hT
            for l in range(n_layers):
                o = l * _PL
                wa = wp[:, o : o + HID]
                wb = wp[:, o + HID : o + 2 * HID]
                Wn1a = wp[:, o + 3 * HID : o + 4 * HID]
                Wn1b = wp[:, o + 4 * HID : o + 5 * HID]
                Wn2 = wp[:, o + 5 * HID : o + 6 * HID]
                c = o + 6 * HID
                be1 = wp[:, c + 1 : c + 2]
                be2 = wp[:, c + 2 : c + 3]
                bn1 = wp[:, c + 3 : c + 4]
                bn2 = wp[:, c + 4 : c + 5]
                bat65 = wp[:, c + 5 : c + 6]
                u_last = -1
                ob = l * _PLB
                We2b = wpb[:, ob : ob + HID]

                # ---- per-layer node projections ----
                ps_ha = psA.tile([HID, N], dt, tag="pre")
                nc.tensor.matmul(ps_ha[:], wa, hT_cur[:], start=True, stop=True)
                haT = lpool.tile([HID, N], dt, tag="haT")
                nc.scalar.activation(haT[:], ps_ha[:], AF.Identity, bias=be1)
                ps_hb = psA.tile([HID, N], dt, tag="pre")
                nc.tensor.matmul(ps_hb[:], wb, hT_cur[:], start=True, stop=True)
                hbT2 = lpool.tile([HID, N2], dt, tag="hbT2")
                nc.vector.tensor_copy(hbT2[:, 0:N], ps_hb[:])
                nc.vector.tensor_copy(hbT2[:, N:N2], ps_hb[:])

                aggT = lpool.tile([HID, N], dt, tag="aggT")
                if n_i < N:
                    nc.vector.memset(aggT[:], 0.0)

                # ---- edge rows: 2-stage software pipeline over groups of
                # 3 pairs. Each iteration emits pre(g), m1(g-1), att/agg(g-2)
                # so every PE matmul's inputs were produced a full group
                # earlier and the PE stream stays dense (HAM warmth).
                groups = [
                    list(range(t0, min(t0 + 3, npair)))
                    for t0 in range(0, npair, 3)
                ]
                stP = None  # (ts, rpre) after pre-stage
                stM = None  # (ts, bs, cs, m) after m1-stage
                for gi in range(len(groups) + 2):
                    ts = groups[gi] if gi < len(groups) else []
                    bs = [32 * (t % 3) for t in ts]
                    cs = [(t // 3) * N2 for t in ts]

                    # stage P: pre matmuls + ha/hb add + relu
                    rpre = {}
                    if ts:
                        ps_pre, pre2 = {}, {}
                        for k, t in enumerate(ts):
                            b, cc = bs[k], cs[k]
                            wc3 = wcr[b : b + 3, l * HID : (l + 1) * HID]
                            ps_pre[k] = psA.tile(
                                [HID, N2], dt, tag="pre", name=f"pspre{k}"
                            )
                            nc.tensor.matmul(
                                ps_pre[k][:],
                                wc3,
                                d2p[b : b + 3, cc : cc + N2],
                                start=True,
                                stop=True,
                            )
                        for k, t in enumerate(ts):
                            pre2[k] = wpool.tile(
                                [HID, N2], dt, tag="pre2", name=f"pre2_{k}"
                            )
                            for h in range(2):
                                i = 2 * t + h
                                nc.vector.scalar_tensor_tensor(
                                    out=pre2[k][:, h * N : (h + 1) * N],
                                    in0=ps_pre[k][:, h * N : (h + 1) * N],
                                    scalar=haT[:, i : i + 1],
                                    in1=hbT2[:, 0:N],
                                    op0=ALU.add,
                                    op1=ALU.add,
                                )
                        for k, t in enumerate(ts):
                            rpre[k] = wpool.tile(
                                [HID, N2], bf, tag="rpre", name=f"rpre{k}", bufs=8
                            )
                            if (gi + k) % 2 == 0:
                                nc.scalar.activation(
                                    rpre[k][:], pre2[k][:], AF.Relu, bias=0.0
                                )
                            else:
                                nc.vector.tensor_scalar_max(
                                    rpre[k][:], pre2[k][:], 0.0
                                )

                    # stage M: edge-MLP second matmul for the previous group
                    m = {}
                    if stP is not None:
                        p_ts, p_rpre = stP
                        ps_m1 = {}
                        for k, t in enumerate(p_ts):
                            ps_m1[k] = psB.tile(
                                [HID, N2], dt, tag="m1", name=f"psm1_{k}"
                            )
                            nc.tensor.matmul(
                                ps_m1[k][:],
                                We2b,
                                p_rpre[k][:],
                                start=True,
                                stop=True,
                            )
                        for k, t in enumerate(p_ts):
                            m[k] = wpool.tile(
                                [HID, N2], bf, tag="m", name=f"m{k}", bufs=8
                            )
                            nc.scalar.activation(
                                m[k][:], ps_m1[k][:], AF.Relu, bias=be2
                            )
                        stM_next = (p_ts, [32 * (t % 3) for t in p_ts],
                                    [(t // 3) * N2 for t in p_ts], m)
                    else:
                        stM_next = None

                    # stage A: attention, sigmoid, gated aggregation for g-2
                    if stM is not None:
                        a_ts, a_bs, a_cs, a_m = stM
                        for k, t in enumerate(a_ts):
                            b, cc = a_bs[k], a_cs[k]
                            WatF = wpb[:, ob + HID : ob + 2 * HID]
                            ps_att = psD.tile(
                                [HID, N2], dt, tag="att", name=f"psatt{k}"
                            )
                            nc.tensor.matmul(
                                ps_att[:], WatF, a_m[k][:], start=True, stop=False
                            )
                            nc.tensor.matmul(
                                ps_att[:],
                                ones1_65[b : b + 1, :],
                                adjp[b : b + 1, cc : cc + N2],
                                start=False,
                                stop=True,
                            )
                            sigp = wpool.tile(
                                [HID, N2], bf, tag="sigp", name=f"sigp{k}"
                            )
                            nc.scalar.activation(
                                sigp[:], ps_att[:], AF.Sigmoid, bias=bat65
                            )
                            for h in range(2):
                                i = 2 * t + h
                                mg = wpool.tile(
                                    [HID, N], bf, tag="mg", name=f"mg{k}{h}"
                                )
                                nc.vector.scalar_tensor_tensor(
                                    out=mg[:],
                                    in0=a_m[k][:, h * N : (h + 1) * N],
                                    scalar=1.0 / NORM,
                                    in1=sigp[:, h * N : (h + 1) * N],
                                    op0=ALU.mult,
                                    op1=ALU.mult,
                                    accum_out=aggT[:, i : i + 1],
                                )
                    stM = stM_next
                    stP = (ts, rpre) if ts else None

                # ---- node MLP:  h += relu([h, agg] @ Wn1 + bn1) @ Wn2 + bn2 ----
                ps_n1 = psA.tile([HID, N], dt, tag="pre")
                nc.tensor.matmul(ps_n1[:], Wn1a, hT_cur[:], start=True, stop=False)
                nc.tensor.matmul(ps_n1[:], Wn1b, aggT[:], start=False, stop=True)
                t1 = wpool.tile([HID, N], dt, tag="pre2")
                nc.scalar.activation(t1[:], ps_n1[:], AF.Relu, bias=bn1)
                ps_n2 = psB.tile([HID, N], dt, tag="m1")
                nc.tensor.matmul(ps_n2[:], Wn2, t1[:], start=True, stop=True)
                hsum = wpool.tile([HID, N], dt, tag="pre2")
                nc.vector.tensor_tensor(hsum[:], ps_n2[:], hT_cur[:], ALU.add)
                hT_new = lpool.tile([HID, N], dt, tag="hT")
                nc.scalar.activation(hT_new[:], hsum[:], AF.Identity, bias=bn2)
                hT_cur = hT_new

            # ---- output head ----
            o = NLAYERS * _PL
            W_out = wp[:, o : o + HID]
            W_lin = wp[:, o + HID : o + HID + 1]
            b_out = wp[:, o + HID + 1 : o + HID + 2]
            b_lin = wp[0:1, o + HID + 2 : o + HID + 3]
            ps_o = psA.tile([HID, NS], dt, tag="pre")
            nc.tensor.matmul(ps_o[:], W_out, hT_cur[:, 0:NS], start=True, stop=True)
            ho = wpool.tile([HID, NS], dt, tag="pre2")
            nc.scalar.activation(ho[:], ps_o[:], AF.Relu, bias=b_out)
            ps_y = psD.tile([1, NS], dt, tag="att")
            nc.tensor.matmul(ps_y[:], W_lin, ho[:], start=True, stop=True)
            y = wpool.tile([1, NS], dt, tag="sig3")
            nc.scalar.activation(y[:], ps_y[:], AF.Identity, bias=b_lin)
            nc.sync.dma_start(out_d.ap(), y[:])


def _make_in_maps(inputs, n_layers, n_i):
    import ml_dtypes

    h0, d2, madj, mask = _host_prep(inputs)
    wp, wcr, wpb = _pack_weights(inputs)
    wcr3 = _pack_wc_split(wcr)
    in_maps = []
    for b in range(B):
        in_maps.append(
            {
                "hT0": np.ascontiguousarray(h0[b].T),
                "d2p": _pack_d2_split(d2[b]),
                "adjp": _pack_pairs3(madj[b], ml_dtypes.bfloat16),
                "wpack": wp,
                "wcrows": wcr3,
                "wpackb": wpb,
            }
        )
    return in_maps, mask


def _install_ntff_hook():
    """Recreate the antenv.axon_hooks module the boot expected, register the
    ctypes NTFF hook from trn_agent_boot, so run_bass_kernel_spmd(trace=True)
    can capture hardware profiles under axon."""
    import types

    if "antenv.axon_hooks" not in sys.modules:
        mod = types.ModuleType("antenv.axon_hooks")
        holder = [None]
        mod.set_axon_ntff_profile_hook = lambda h: holder.__setitem__(0, h)
        mod.get_axon_ntff_profile_hook = lambda: holder[0]
        sys.modules["antenv.axon_hooks"] = mod
        import antenv

        antenv.axon_hooks = mod
    m = sys.modules["antenv.axon_hooks"]
    if m.get_axon_ntff_profile_hook() is None:
        sys.path.insert(0, "/root/.axon_site")
        from trn_agent_boot.trn_boot import _ntff_profile_via_ctypes

        m.set_axon_ntff_profile_hook(
            _ntff_profile_via_ctypes("/opt/axon/libaxon_pjrt.so")
        )


_CACHE = {}


def _get_nc(n_layers, n_i):
    key = (n_layers, n_i)
    if key not in _CACHE:
        import concourse.bass as bass
        import concourse.tile as tile
        from concourse import bacc

        nc = bacc.Bacc(
            "TRN2", target_bir_lowering=False, debug=False, num_devices=B
        )
        _build(nc, tile, bass, n_layers, n_i)
        nc.compile()
        _CACHE[key] = nc
    return _CACHE[key]


def kernel(**inputs):
    inputs = {k: np.asarray(v) for k, v in inputs.items()}
    n_layers = int(os.environ.get("GNN_LAYERS", NLAYERS))
    n_i = int(os.environ.get("GNN_NI", N))
    in_maps, mask = _make_in_maps(inputs, n_layers, n_i)
    nc = _get_nc(n_layers, n_i)

    if os.environ.get("GNN_SIM"):
        from concourse.bass_interp import CoreSim

        sim = CoreSim(nc, trace=False)
        outs = []
        for b in range(int(os.environ.get("GNN_SIM_CORES", 1))):
            for k, v in in_maps[b].items():
                sim.tensor(k)[:] = v
            sim.simulate()
            outs.append(np.array(sim.tensor("out")).reshape(NS, 1))
        while len(outs) < B:
            outs.append(np.zeros((NS, 1), _F32))
        out = np.stack(outs)
    else:
        from concourse.bass_utils import run_bass_kernel_spmd

        if os.environ.get("GNN_TRACE"):
            _install_ntff_hook()
            tmpdir = os.environ.get("GNN_TRACE_DIR") or None
            try:
                res = run_bass_kernel_spmd(
                    nc, in_maps, core_ids=list(range(B)), trace=True, tmpdir=tmpdir
                )
                kernel.last_exec_time_ns = res.exec_time_ns
            except Exception as e:
                print(f"[gnn] traced run failed ({e!r}); retrying untraced")
                res = run_bass_kernel_spmd(nc, in_maps, core_ids=list(range(B)))
        else:
            res = run_bass_kernel_spmd(nc, in_maps, core_ids=list(range(B)))
        kernel.last_results = res
        out = np.stack([r["out"].reshape(NS, 1) for r in res.results])

    return (out * inputs["node_mask"][:, :, None]).astype(_F32)



# revision 16
# speedup vs baseline: 5.9129x; 5.9129x over previous
"""Trainium2 Bass kernel for AnchorGNNPocket (GNN message passing), sparse-edge
formulation.

Data-parallel over batch B=8: one complex per NeuronCore. The cutoff graph is
~12% dense (max 8688 of 65536 edges), so instead of the dense [N,N] edge MLP we
pack the active edges into E_cap=8704 columns and run the whole edge pipeline
on [128, E] tiles:

- Host extracts the active edge list (i_e, j_e, d2_e) per sample, builds
  one-hot gather matrices Si/Sj (ha[i_e]/hb[j_e] broadcast becomes a PE
  matmul), the d2 hi/lo rows for a K=3 bf16 matmul (fp32-class accuracy), and
  the 0/1 scatter matrix A (padding edges have all-zero rows -> contribute 0).
- Per 512-edge tile: one 5-matmul PSUM accumulation group builds
  pre[h, e] = ha[:,i_e] + hb[:,j_e] + wc*d2_e; relu(+be1) is a single ScalarE
  activation (be1 is per-partition in this layout).
- Per 128-edge chunk: the second edge-MLP matmul uses rpre as the STATIONARY
  operand so m1 comes out [e-part, h-free]; then be2-add (gpsimd), relu
  (ScalarE), attention logit via DVE mult-reduce against a replicated Wat
  (accum_out), batched sigmoid, gate via tensor_scalar_mul, and a scatter
  matmul (m_g stationary, A chunk moving) accumulating all chunks into one
  [128, 256] PSUM tile = aggT.
- 1/NORM is folded into Wn1b on the host. Node MLP and output head are dense
  and cheap.
"""

import os
import sys

import numpy as np

if not any(os.path.isdir(os.path.join(p, "concourse")) for p in sys.path if p):
    sys.path.insert(0, "/opt/trn_rl_repo")

# ---- problem constants (hardcoded per contest rules) ----
B, NS, NP = 8, 32, 224
N = NS + NP                      # 256 nodes
LIG_NF, POK_NF, JNF, HID, OUT_NF, NLAYERS = 10, 25, 32, 128, 128, 4
CUT2 = 4.5 ** 2
NORM = 100.0

_F32 = np.float32
E_CAP = 8704                     # 17 tiles x 512 = 68 chunks x 128
NTILE = E_CAP // 512
NCHUNK = E_CAP // 128


def _np_silu(x):
    return x / (1.0 + np.exp(-x))


def _host_prep(inputs):
    """Embedding h0, pairwise d2, adjacency -> per-sample packed edge data."""
    x = np.concatenate([inputs["mol_x"], inputs["pocket_x"]], axis=1).astype(_F32)
    mask = np.concatenate([inputs["node_mask"], inputs["pocket_mask"]], axis=1).astype(
        _F32
    )
    hm = _np_silu(inputs["mol_h"].astype(_F32) @ inputs["W_mol"] + inputs["b_mol"])
    hp = _np_silu(
        inputs["pocket_h"].astype(_F32) @ inputs["W_pok"] + inputs["b_pok"]
    )
    h0 = (
        np.concatenate([hm, hp], axis=1) @ inputs["W_emb"] + inputs["b_emb"]
    ).astype(_F32)  # [B, N, H]

    diff = x[:, :, None, :] - x[:, None, :, :]
    d2 = np.sum(diff * diff, axis=-1, dtype=_F32)  # [B, N, N]
    idx = np.arange(N)
    lig_pair = (idx[:, None] < NS) & (idx[None, :] < NS)
    adj = np.where(lig_pair, True, d2 <= CUT2)
    adj = adj & (mask[:, :, None] > 0) & (mask[:, None, :] > 0)
    return h0, d2, adj, mask


def _pack_edges(d2_s, adj_s):
    """One sample's graph -> (Si0, Si1, Sj0, Sj1, d2p3, A)."""
    import ml_dtypes

    bf = ml_dtypes.bfloat16
    ii, jj = np.nonzero(adj_s)
    E = ii.shape[0]
    assert E <= E_CAP, f"edge capacity exceeded: {E} > {E_CAP}"
    Si0 = np.zeros((HID, E_CAP), dtype=bf)
    Si1 = np.zeros((HID, E_CAP), dtype=bf)
    Sj0 = np.zeros((HID, E_CAP), dtype=bf)
    Sj1 = np.zeros((HID, E_CAP), dtype=bf)
    e = np.arange(E)
    lo = ii < HID
    Si0[ii[lo], e[lo]] = 1.0
    Si1[ii[~lo] - HID, e[~lo]] = 1.0
    lo = jj < HID
    Sj0[jj[lo], e[lo]] = 1.0
    Sj1[jj[~lo] - HID, e[~lo]] = 1.0

    dvals = np.zeros((E_CAP,), dtype=_F32)
    dvals[:E] = d2_s[ii, jj]
    hi = dvals.astype(bf)
    lop = (dvals - hi.astype(_F32)).astype(bf)
    d2p3 = np.stack([hi, lop, hi])  # [3, E_CAP] rows pair lhsT [wc_hi, wc_lo, wc_hi]

    A = np.zeros((128, NCHUNK * N), dtype=bf)
    A[e % 128, (e // 128) * N + ii] = 1.0
    return Si0, Si1, Sj0, Sj1, d2p3, A


# weight-pack column offsets
_PL = 3 * HID + 4                       # Wn1a, Wn1b/NORM, Wn2 + be1, bn1, bn2, bat
_W_COLS = NLAYERS * _PL + HID + 3       # + W_out + wlin/bout/blin
_PLB = 4 * HID                          # bf16 per layer: wa | wb | We2 | WatB
_WB_COLS = NLAYERS * _PLB


def _pack_weights(inputs):
    import ml_dtypes

    bf = ml_dtypes.bfloat16
    wp = np.zeros((HID, _W_COLS), dtype=_F32)
    wpb = np.zeros((HID, _WB_COLS), dtype=bf)
    wc3 = np.zeros((3, NLAYERS * HID), dtype=bf)
    becb = np.zeros((HID, NLAYERS * 512), dtype=_F32)
    bats = np.zeros((NLAYERS,), dtype=_F32)
    be2z = np.zeros((NLAYERS,), dtype=bool)
    We1 = inputs["We1"].astype(_F32)
    for l in range(NLAYERS):
        o = l * _PL
        wp[:, o : o + HID] = inputs["Wn1"][l][:HID, :]
        wp[:, o + HID : o + 2 * HID] = inputs["Wn1"][l][HID:, :] / NORM
        wp[:, o + 2 * HID : o + 3 * HID] = inputs["Wn2"][l]
        c = o + 3 * HID
        wp[:, c + 0] = inputs["be1"][l]
        wp[:, c + 1] = inputs["bn1"][l]
        wp[:, c + 2] = inputs["bn2"][l]
        # c+3 free
        ob = l * _PLB
        wpb[:, ob : ob + HID] = We1[l, :HID, :]                 # wa
        wpb[:, ob + HID : ob + 2 * HID] = We1[l, HID : 2 * HID, :]  # wb
        wpb[:, ob + 2 * HID : ob + 3 * HID] = inputs["We2"][l]
        wpb[:, ob + 3 * HID : ob + 4 * HID] = np.repeat(
            inputs["Wat"][l].astype(_F32).T, HID, axis=0
        )  # WatB[p, h] = Wat[h]
        wcr = We1[l, 2 * HID, :].astype(_F32)
        whi = wcr.astype(bf)
        wlo = (wcr - whi.astype(_F32)).astype(bf)
        # pairs with d2p3 rows [hi, lo, hi]: whi*hi + whi*lo + wlo*hi
        wc3[0, l * HID : (l + 1) * HID] = whi
        wc3[1, l * HID : (l + 1) * HID] = whi
        wc3[2, l * HID : (l + 1) * HID] = wlo
        becb[:, l * 512 : (l + 1) * 512] = np.tile(inputs["be2"][l], 4)[None, :]
        bats[l] = float(np.asarray(inputs["bat"][l]).reshape(-1)[0])
        be2z[l] = not np.any(np.asarray(inputs["be2"][l]))
    o = NLAYERS * _PL
    wp[:, o : o + HID] = inputs["W_out"].astype(_F32)
    wp[:, o + HID] = inputs["W_lin"][:, 0]
    wp[:, o + HID + 1] = inputs["b_out"]
    wp[0, o + HID + 2] = inputs["b_lin"][0]
    return wp, wpb, wc3, becb, bats, be2z


def _build(nc, tile_mod, bass_mod, n_layers, bats, be2z):
    """Trace the per-core sparse kernel into nc (a Bacc)."""
    mybir = __import__("concourse.mybir", fromlist=["mybir"])
    dt = mybir.dt.float32
    bf = mybir.dt.bfloat16
    AF = mybir.ActivationFunctionType
    ALU = mybir.AluOpType

    hT_d = nc.dram_tensor("hT0", [HID, N], dt, kind="ExternalInput")
    hTb_d = nc.dram_tensor("hT0b", [HID, N], bf, kind="ExternalInput")
    si0_d = nc.dram_tensor("Si0", [HID, E_CAP], bf, kind="ExternalInput")
    si1_d = nc.dram_tensor("Si1", [HID, E_CAP], bf, kind="ExternalInput")
    sj0_d = nc.dram_tensor("Sj0", [HID, E_CAP], bf, kind="ExternalInput")
    sj1_d = nc.dram_tensor("Sj1", [HID, E_CAP], bf, kind="ExternalInput")
    d2_d = nc.dram_tensor("d2p3", [3, E_CAP], bf, kind="ExternalInput")
    a_d = nc.dram_tensor("Ascat", [128, NCHUNK * N], bf, kind="ExternalInput")
    wp_d = nc.dram_tensor("wpack", [HID, _W_COLS], dt, kind="ExternalInput")
    wb_d = nc.dram_tensor("wpackb", [HID, _WB_COLS], bf, kind="ExternalInput")
    wc_d = nc.dram_tensor("wc3", [3, NLAYERS * HID], bf, kind="ExternalInput")
    bec_d = nc.dram_tensor("becb", [HID, NLAYERS * 512], dt, kind="ExternalInput")
    out_d = nc.dram_tensor("out", [1, NS], dt, kind="ExternalOutput")

    with tile_mod.TileContext(nc) as tc:
        with (
            tc.tile_pool(name="const", bufs=1) as cpool,
            tc.tile_pool(name="layer", bufs=2) as lpool,
            tc.tile_pool(name="work", bufs=4) as wpool,
            tc.tile_pool(name="psA", bufs=3, space="PSUM") as psA,
            tc.tile_pool(name="psB", bufs=2, space="PSUM") as psB,
            tc.tile_pool(name="psC", bufs=1, space="PSUM") as psC,
        ):
            # ---- load constants ----
            hT = cpool.tile([HID, N], dt, tag="hT0")
            hTb0 = cpool.tile([HID, N], bf, tag="hT0b")
            Si0 = cpool.tile([HID, E_CAP], bf, tag="Si0")
            Si1 = cpool.tile([HID, E_CAP], bf, tag="Si1")
            Sj0 = cpool.tile([HID, E_CAP], bf, tag="Sj0")
            Sj1 = cpool.tile([HID, E_CAP], bf, tag="Sj1")
            d2p = cpool.tile([3, E_CAP], bf, tag="d2p3")
            Asc = cpool.tile([128, NCHUNK * N], bf, tag="Ascat")
            wp = cpool.tile([HID, _W_COLS], dt, tag="wpack")
            wpb = cpool.tile([HID, _WB_COLS], bf, tag="wpackb")
            wc3 = cpool.tile([3, NLAYERS * HID], bf, tag="wc3")
            becb = cpool.tile([HID, NLAYERS * 512], dt, tag="becb")
            for t, d in (
                (hT, hT_d), (hTb0, hTb_d), (wp, wp_d), (wpb, wb_d),
                (wc3, wc_d), (becb, bec_d), (d2p, d2_d),
                (Si0, si0_d), (Si1, si1_d), (Sj0, sj0_d), (Sj1, sj1_d),
                (Asc, a_d),
            ):
                nc.sync.dma_start(t[:], d.ap())

            hT_cur, hTb_cur = hT, hTb0
            for l in range(n_layers):
                o = l * _PL
                Wn1a = wp[:, o : o + HID]
                Wn1b = wp[:, o + HID : o + 2 * HID]
                Wn2 = wp[:, o + 2 * HID : o + 3 * HID]
                c = o + 3 * HID
                be1 = wp[:, c + 0 : c + 1]
                bn1 = wp[:, c + 1 : c + 2]
                bn2 = wp[:, c + 2 : c + 3]
                ob = l * _PLB
                wab = wpb[:, ob : ob + HID]
                wbb = wpb[:, ob + HID : ob + 2 * HID]
                We2b = wpb[:, ob + 2 * HID : ob + 3 * HID]
                WatB = wpb[:, ob + 3 * HID : ob + 4 * HID]
                wc3l = wc3[:, l * HID : (l + 1) * HID]
                be2b = becb[:, l * 512 : (l + 1) * 512]
                be2_zero = bool(be2z[l])

                # ---- ha_rows/hb_rows: [i-part, h-free], bf16 ----
                ps_h0 = psA.tile([HID, 2 * HID], dt, tag="pre")
                nc.tensor.matmul(ps_h0[:, 0:HID], hTb_cur[:, 0:HID], wab,
                                 start=True, stop=True)
                nc.tensor.matmul(ps_h0[:, HID : 2 * HID], hTb_cur[:, 0:HID], wbb,
                                 start=True, stop=True)
                ps_h1 = psA.tile([HID, 2 * HID], dt, tag="pre")
                nc.tensor.matmul(ps_h1[:, 0:HID], hTb_cur[:, HID:N], wab,
                                 start=True, stop=True)
                nc.tensor.matmul(ps_h1[:, HID : 2 * HID], hTb_cur[:, HID:N], wbb,
                                 start=True, stop=True)
                har0 = lpool.tile([HID, 2 * HID], bf, tag="har0")
                har1 = lpool.tile([HID, 2 * HID], bf, tag="har1")
                nc.vector.tensor_copy(har0[:], ps_h0[:])
                nc.vector.tensor_copy(har1[:], ps_h1[:])

                attc = lpool.tile([HID, NCHUNK], dt, tag="attc")
                sigc = lpool.tile([HID, NCHUNK], dt, tag="sigc")
                ps_agg = psC.tile([HID, N], dt, tag="agg")

                # ---- edge tiles ----
                for t in range(NTILE):
                    sl = slice(t * 512, (t + 1) * 512)
                    ps_pre = psA.tile([HID, 512], dt, tag="pre", name=f"pre{t}")
                    nc.tensor.matmul(ps_pre[:], har0[:, 0:HID], Si0[:, sl],
                                     start=True, stop=False)
                    nc.tensor.matmul(ps_pre[:], har1[:, 0:HID], Si1[:, sl],
                                     start=False, stop=False)
                    nc.tensor.matmul(ps_pre[:], har0[:, HID : 2 * HID], Sj0[:, sl],
                                     start=False, stop=False)
                    nc.tensor.matmul(ps_pre[:], har1[:, HID : 2 * HID], Sj1[:, sl],
                                     start=False, stop=False)
                    nc.tensor.matmul(ps_pre[:], wc3l, d2p[:, sl],
                                     start=False, stop=True)
                    rpre = wpool.tile([HID, 512], bf, tag="rpre", bufs=4)
                    nc.scalar.activation(rpre[:], ps_pre[:], AF.Relu, bias=be1)

                    ps_m1 = psB.tile([128, 512], dt, tag="m1", name=f"m1_{t}")
                    for k in range(4):
                        ck = slice(k * HID, (k + 1) * HID)
                        nc.tensor.matmul(ps_m1[:, ck], rpre[:, ck], We2b,
                                         start=True, stop=True)
                    m = wpool.tile([128, 512], bf, tag="m", bufs=4)
                    if be2_zero:
                        nc.scalar.activation(m[:], ps_m1[:], AF.Relu, bias=0.0)
                    else:
                        m1s = wpool.tile([128, 512], dt, tag="m1s", bufs=3)
                        nc.vector.tensor_tensor(m1s[:], ps_m1[:], be2b, ALU.add)
                        nc.scalar.activation(m[:], m1s[:], AF.Relu, bias=0.0)

                    scr = wpool.tile([128, 512], bf, tag="scr", bufs=2)
                    for k in range(4):
                        ck = slice(k * HID, (k + 1) * HID)
                        cc = t * 4 + k
                        nc.vector.scalar_tensor_tensor(
                            out=scr[:, ck], in0=m[:, ck], scalar=1.0,
                            in1=WatB, op0=ALU.mult, op1=ALU.mult,
                            accum_out=attc[:, cc : cc + 1],
                        )
                    nc.scalar.activation(
                        sigc[:, t * 4 : (t + 1) * 4], attc[:, t * 4 : (t + 1) * 4],
                        AF.Sigmoid, bias=float(bats[l]),
                    )
                    mg = wpool.tile([128, 512], bf, tag="mg", bufs=4)
                    for k in range(4):
                        ck = slice(k * HID, (k + 1) * HID)
                        cc = t * 4 + k
                        nc.vector.tensor_scalar_mul(
                            mg[:, ck], m[:, ck], sigc[:, cc : cc + 1]
                        )
                    for k in range(4):
                        ck = slice(k * HID, (k + 1) * HID)
                        cc = t * 4 + k
                        nc.tensor.matmul(
                            ps_agg[:], mg[:, ck], Asc[:, cc * N : (cc + 1) * N],
                            start=(t == 0 and k == 0),
                            stop=(t == NTILE - 1 and k == 3),
                        )

                # ---- node MLP:  h += relu([h, agg] @ Wn1 + bn1) @ Wn2 + bn2 ----
                aggT = wpool.tile([HID, N], dt, tag="m1s")
                nc.vector.tensor_copy(aggT[:], ps_agg[:])
                ps_n1 = psA.tile([HID, N], dt, tag="pre")
                nc.tensor.matmul(ps_n1[:], Wn1a, hT_cur[:], start=True, stop=False)
                nc.tensor.matmul(ps_n1[:], Wn1b, aggT[:], start=False, stop=True)
                t1 = wpool.tile([HID, N], dt, tag="m1s")
                nc.scalar.activation(t1[:], ps_n1[:], AF.Relu, bias=bn1)
                ps_n2 = psB.tile([HID, N], dt, tag="m1")
                nc.tensor.matmul(ps_n2[:], Wn2, t1[:], start=True, stop=True)
                hsum = wpool.tile([HID, N], dt, tag="m1s")
                nc.vector.tensor_tensor(hsum[:], ps_n2[:], hT_cur[:], ALU.add)
                hT_new = lpool.tile([HID, N], dt, tag="hT")
                nc.scalar.activation(hT_new[:], hsum[:], AF.Identity, bias=bn2)
                hTb_new = lpool.tile([HID, N], bf, tag="hTb")
                nc.vector.tensor_copy(hTb_new[:], hT_new[:])
                hT_cur, hTb_cur = hT_new, hTb_new

            # ---- output head ----
            o = NLAYERS * _PL
            W_out = wp[:, o : o + HID]
            W_lin = wp[:, o + HID : o + HID + 1]
            b_out = wp[:, o + HID + 1 : o + HID + 2]
            b_lin = wp[0:1, o + HID + 2 : o + HID + 3]
            ps_o = psA.tile([HID, NS], dt, tag="pre")
            nc.tensor.matmul(ps_o[:], W_out, hT_cur[:, 0:NS], start=True, stop=True)
            ho = wpool.tile([HID, NS], dt, tag="m1s")
            nc.scalar.activation(ho[:], ps_o[:], AF.Relu, bias=b_out)
            ps_y = psB.tile([1, NS], dt, tag="m1")
            nc.tensor.matmul(ps_y[:], W_lin, ho[:], start=True, stop=True)
            y = wpool.tile([1, NS], dt, tag="scr", bufs=2)
            nc.scalar.activation(y[:], ps_y[:], AF.Identity, bias=b_lin)
            nc.sync.dma_start(out_d.ap(), y[:])


def _make_in_maps(inputs, n_layers):
    import ml_dtypes

    bf = ml_dtypes.bfloat16
    h0, d2, adj, mask = _host_prep(inputs)
    wp, wpb, wc3, becb, bats, be2z = _pack_weights(inputs)
    in_maps = []
    for b in range(B):
        Si0, Si1, Sj0, Sj1, d2p3, A = _pack_edges(d2[b], adj[b])
        hTb = np.ascontiguousarray(h0[b].T)
        in_maps.append(
            {
                "hT0": hTb,
                "hT0b": hTb.astype(bf),
                "Si0": Si0, "Si1": Si1, "Sj0": Sj0, "Sj1": Sj1,
                "d2p3": d2p3, "Ascat": A,
                "wpack": wp, "wpackb": wpb, "wc3": wc3, "becb": becb,
            }
        )
    return in_maps, mask, bats, be2z


def _install_ntff_hook():
    """Recreate the antenv.axon_hooks module the boot expected, register the
    ctypes NTFF hook from trn_agent_boot, so run_bass_kernel_spmd(trace=True)
    can capture hardware profiles under axon."""
    import types

    if "antenv.axon_hooks" not in sys.modules:
        mod = types.ModuleType("antenv.axon_hooks")
        holder = [None]
        mod.set_axon_ntff_profile_hook = lambda h: holder.__setitem__(0, h)
        mod.get_axon_ntff_profile_hook = lambda: holder[0]
        sys.modules["antenv.axon_hooks"] = mod
        import antenv

        antenv.axon_hooks = mod
    m = sys.modules["antenv.axon_hooks"]
    if m.get_axon_ntff_profile_hook() is None:
        sys.path.insert(0, "/root/.axon_site")
        from trn_agent_boot.trn_boot import _ntff_profile_via_ctypes

        m.set_axon_ntff_profile_hook(
            _ntff_profile_via_ctypes("/opt/axon/libaxon_pjrt.so")
        )


_CACHE = {}


def _get_nc(n_layers, bats, be2z):
    key = (n_layers, tuple(np.round(bats, 8)), tuple(be2z))
    if key not in _CACHE:
        import concourse.bass as bass
        import concourse.tile as tile
        from concourse import bacc

        nc = bacc.Bacc(
            "TRN2", target_bir_lowering=False, debug=False, num_devices=B
        )
        _build(nc, tile, bass, n_layers, bats, be2z)
        nc.compile()
        _CACHE[key] = nc
    return _CACHE[key]


def kernel(**inputs):
    inputs = {k: np.asarray(v) for k, v in inputs.items()}
    n_layers = int(os.environ.get("GNN_LAYERS", NLAYERS))
    in_maps, mask, bats, be2z = _make_in_maps(inputs, n_layers)
    nc = _get_nc(n_layers, bats, be2z)

    if os.environ.get("GNN_SIM"):
        from concourse.bass_interp import CoreSim

        outs = []
        for b in range(int(os.environ.get("GNN_SIM_CORES", 1))):
            sim = CoreSim(nc, trace=False)
            for k, v in in_maps[b].items():
                sim.tensor(k)[:] = v
            sim.simulate()
            outs.append(np.array(sim.tensor("out")).reshape(NS, 1))
        while len(outs) < B:
            outs.append(np.zeros((NS, 1), _F32))
        out = np.stack(outs)
    else:
        from concourse.bass_utils import run_bass_kernel_spmd

        if os.environ.get("GNN_TRACE"):
            _install_ntff_hook()
            tmpdir = os.environ.get("GNN_TRACE_DIR") or None
            try:
                res = run_bass_kernel_spmd(
                    nc, in_maps, core_ids=list(range(B)), trace=True, tmpdir=tmpdir
                )
                kernel.last_exec_time_ns = res.exec_time_ns
            except Exception as e:
                print(f"[gnn] traced run failed ({e!r}); retrying untraced")
                res = run_bass_kernel_spmd(nc, in_maps, core_ids=list(range(B)))
        else:
            res = run_bass_kernel_spmd(nc, in_maps, core_ids=list(range(B)))
        kernel.last_results = res
        out = np.stack([r["out"].reshape(NS, 1) for r in res.results])

    return (out * inputs["node_mask"][:, :, None]).astype(_F32)
